# revision 5
# baseline (speedup 1.0000x reference)
"""Block-diagonal 4-layer MLP (8 experts) on 8 Trainium2 NeuronCores.

Expert-parallel: core e computes expert e's chain
    h = relu(W0_e @ x.T + b0_e); h = relu(W1_e @ h + b1_e);
    h = relu(W2_e @ h + b2_e);   y_e.T = W3_e @ h + b3_e
with activations stored transposed [features, batch] so the tensor engine
streams batch as the moving free dim. Weights are fed pre-transposed
(W_e.T = [in, out]) so lhsT tiles slice directly. fp32r matmuls (full PE
rate), bias+ReLU fused into one ScalarE/VectorE op reading PSUM.
"""

import sys

import numpy as np

for _p in ("/opt/trn_rl_repo", "/root/.axon_site/_ro/trn_rl_repo"):
    if _p not in sys.path:
        sys.path.append(_p)

import concourse.bass as bass  # noqa: E402
import concourse.tile as tile  # noqa: E402
from concourse import bacc, mybir  # noqa: E402
from concourse.bass_utils import run_bass_kernel_spmd  # noqa: E402

N_PAR = 8
IN, HID, OUT, B = 256, 512, 256, 1024
P = 128
BN = 512  # batch chunk = max fp32 moving free dim = one PSUM bank
NB = B // BN
F32 = mybir.dt.float32
F32R = mybir.dt.float32r
# (K, M) of each layer's W^T
DIMS = [(IN, HID), (HID, HID), (HID, HID), (HID, OUT)]

_cached_nc = None
LAST_RESULTS = None


def _build():
    nc = bacc.Bacc(
        trn_type="TRN2",
        target_bir_lowering=False,
        debug=False,
        num_devices=N_PAR,
    )
    xt = nc.dram_tensor("xt", [IN, B], F32R, kind="ExternalInput").ap()
    w_aps = [
        nc.dram_tensor(f"w{l}t", [k, m], F32R, kind="ExternalInput").ap()
        for l, (k, m) in enumerate(DIMS)
    ]
    b_aps = [
        nc.dram_tensor(f"b{l}", [m], F32, kind="ExternalInput").ap()
        for l, (_, m) in enumerate(DIMS)
    ]
    yt = nc.dram_tensor("yt", [OUT, B], F32, kind="ExternalOutput").ap()
    yt_t = yt.rearrange("(mt p) b -> p mt b", p=P)

    with tile.TileContext(nc) as tc:
        with (
            tc.tile_pool(name="w", bufs=1) as wpool,
            tc.tile_pool(name="acts", bufs=1) as apool,
            tc.tile_pool(name="outs", bufs=4) as opool,
            tc.tile_pool(name="psum", bufs=8, space="PSUM") as psum,
        ):
            x_sb = apool.tile([P, IN // P, B], F32R, tag="x")
            nc.sync.dma_start(x_sb[:], xt.rearrange("(kt p) b -> p kt b", p=P))

            w_sb, b_sb = [], []
            for l, (k, m) in enumerate(DIMS):
                wt = wpool.tile([P, k // P, m], F32R, tag=f"w{l}")
                nc.sync.dma_start(wt[:], w_aps[l].rearrange("(kt p) m -> p kt m", p=P))
                w_sb.append(wt)
                bt = wpool.tile([P, m // P], F32, tag=f"b{l}")
                nc.sync.dma_start(bt[:], b_aps[l].rearrange("(mt p) -> p mt", p=P))
                b_sb.append(bt)

            act = x_sb
            for l, (k_dim, m_dim) in enumerate(DIMS):
                kt, mt = k_dim // P, m_dim // P
                last = l == len(DIMS) - 1
                h = None if last else apool.tile([P, mt, B], F32R, tag=f"h{l}")
                idx = 0
                for m in range(mt):
                    bias = b_sb[l][:, m : m + 1]
                    for n in range(NB):
                        ps = psum.tile([P, BN], F32, tag="ps")
                        for k in range(kt):
                            nc.tensor.matmul(
                                ps[:],
                                w_sb[l][:, k, m * P : (m + 1) * P],
                                act[:, k, n * BN : (n + 1) * BN],
                                start=(k == 0),
                                stop=(k == kt - 1),
                            )
                        on_scalar = idx % 2 == 0
                        if last:
                            o = opool.tile([P, BN], F32, tag="o")
                            if on_scalar:
                                nc.scalar.activation(
                                    o[:],
                                    ps[:],
                                    mybir.ActivationFunctionType.Identity,
                                    bias=bias,
                                )
                            else:
                                nc.vector.tensor_scalar(
                                    o[:], ps[:], bias, None, mybir.AluOpType.add
                                )
                            nc.sync.dma_start(
                                yt_t[:, m, n * BN : (n + 1) * BN], o[:]
                            )
                        else:
                            dst = h[:, m, n * BN : (n + 1) * BN]
                            if on_scalar:
                                nc.scalar.activation(
                                    dst,
                                    ps[:],
                                    mybir.ActivationFunctionType.Relu,
                                    bias=bias,
                                )
                            else:
                                nc.vector.tensor_scalar(
                                    dst,
                                    ps[:],
                                    bias,
                                    0.0,
                                    mybir.AluOpType.add,
                                    mybir.AluOpType.max,
                                )
                        idx += 1
                act = h
    nc.compile()
    return nc


def kernel(_trace=False, **inputs):
    global _cached_nc, LAST_RESULTS
    x = np.ascontiguousarray(inputs["x"], dtype=np.float32)
    if _cached_nc is None:
        _cached_nc = _build()
    nc = _cached_nc

    xt = np.ascontiguousarray(x.T)
    out_sizes = [HID, HID, HID, OUT]
    in_sizes = [IN, HID, HID, HID]
    in_maps = []
    for e in range(N_PAR):
        m = {"xt": xt}
        for l in range(4):
            r0, c0 = e * out_sizes[l], e * in_sizes[l]
            blk = inputs[f"W{l}"][r0 : r0 + out_sizes[l], c0 : c0 + in_sizes[l]]
            m[f"w{l}t"] = np.ascontiguousarray(np.asarray(blk).T, dtype=np.float32)
            m[f"b{l}"] = np.ascontiguousarray(
                np.asarray(inputs[f"b{l}"][r0 : r0 + out_sizes[l]]), dtype=np.float32
            )
        in_maps.append(m)

    res = run_bass_kernel_spmd(
        nc, in_maps, core_ids=list(range(N_PAR)), trace=_trace
    )
    LAST_RESULTS = res
    y_p = np.concatenate(
        [res.results[e]["yt"].T for e in range(N_PAR)], axis=1
    ).astype(np.float32)
    x_p = np.tile(x, (1, N_PAR)).astype(np.float32)
    return (y_p, x_p)


# revision 30
# speedup vs baseline: 1.3460x; 1.3460x over previous
"""Block-diagonal 4-layer MLP (8 experts) on 8 Trainium2 NeuronCores.

Expert-parallel: core e computes expert e's chain
    h = relu(W0_e @ x.T + b0_e); h = relu(W1_e @ h + b1_e);
    h = relu(W2_e @ h + b2_e);   y_e.T = W3_e @ h + b3_e
with activations stored transposed [features, batch] so the tensor engine
streams batch as the moving free dim. Weights are fed pre-transposed
(W_e.T = [in, out]) so lhsT tiles slice directly. fp32r matmuls (full PE
rate), bias+ReLU fused into one ScalarE/VectorE op reading PSUM.

Startup: dummy matmuls on a zeroed SBUF tile keep the PE array busy (HAM
warm) while input DMAs stream in, and a dummy ReLU preloads the ACT
function table. Input DMAs are chunked and ordered so the first batch
half's L0 tiles unblock as early as possible.
"""

import sys

import numpy as np

for _p in ("/opt/trn_rl_repo", "/root/.axon_site/_ro/trn_rl_repo"):
    if _p not in sys.path:
        sys.path.append(_p)

import concourse.bass as bass  # noqa: E402
import concourse.tile as tile  # noqa: E402
from concourse import bacc, mybir  # noqa: E402
from concourse.bass_utils import run_bass_kernel_spmd  # noqa: E402

N_PAR = 8
IN, HID, OUT, B = 256, 512, 256, 1024
P = 128
BN = 512  # batch chunk = max fp32 moving free dim = one PSUM bank
NB = B // BN
F32 = mybir.dt.float32
F32R = mybir.dt.float32r
# (K, M) of each layer's W^T
DIMS = [(IN, HID), (HID, HID), (HID, HID), (HID, OUT)]
WARMUP_MMS = 6

_cached_nc = None
LAST_RESULTS = None


def _build(warmup_mms=WARMUP_MMS, l3_parity=0, fill1=1, fill2=2,
           l0_order=1, l3_interleave=False):
    nc = bacc.Bacc(
        trn_type="TRN2",
        target_bir_lowering=False,
        debug=False,
        num_devices=N_PAR,
    )
    xt = nc.dram_tensor("xt", [IN, B], F32R, kind="ExternalInput").ap()
    w_aps = [
        nc.dram_tensor(f"w{l}t", [k, m], F32R, kind="ExternalInput").ap()
        for l, (k, m) in enumerate(DIMS)
    ]
    b_aps = [
        nc.dram_tensor(f"b{l}", [m], F32, kind="ExternalInput").ap()
        for l, (_, m) in enumerate(DIMS)
    ]
    yt = nc.dram_tensor("yt", [OUT, B], F32, kind="ExternalOutput").ap()
    yt_t = yt.rearrange("(mt p) b -> p mt b", p=P)

    with tile.TileContext(nc) as tc:
        with (
            tc.tile_pool(name="w", bufs=1) as wpool,
            tc.tile_pool(name="acts", bufs=1) as apool,
            tc.tile_pool(name="outs", bufs=4) as opool,
            tc.tile_pool(name="psum", bufs=6, space="PSUM") as psum,
            tc.tile_pool(name="warm", bufs=1, space="PSUM") as warmpool,
        ):
            # --- PE warmup + ACT table preload (no DMA dependency) ---
            # bf16 dummy matmuls: same PE streaming rate as fp32r, no
            # fp32r-producer rounding constraint on the memset
            warm_src = apool.tile([P, BN], mybir.dt.bfloat16, tag="warmsrc")
            nc.vector.memset(warm_src[:], 0.0)
            warm_ps = warmpool.tile([P, BN], F32, tag="warmps")
            for _ in range(warmup_mms):
                nc.tensor.matmul(
                    warm_ps[:], warm_src[:, :P], warm_src[:], start=True, stop=True
                )
            warm_act = apool.tile([P, 1], F32, tag="warmact")
            nc.scalar.activation(
                warm_act[:], warm_src[:, :1],
                mybir.ActivationFunctionType.Relu,
            )

            # --- SBUF allocations ---
            x_sb = apool.tile([P, IN // P, B], F32R, tag="x")
            w_sb = [
                wpool.tile([P, k // P, m], F32R, tag=f"w{l}", name=f"w{l}")
                for l, (k, m) in enumerate(DIMS)
            ]
            b_sb = [
                wpool.tile([P, m // P], F32, tag=f"b{l}", name=f"b{l}")
                for l, (_, m) in enumerate(DIMS)
            ]
            h_sb = [
                apool.tile([P, m // P, B], F32R, tag=f"h{l}", name=f"h{l}")
                for l, (_, m) in enumerate(DIMS[:-1])
            ]

            # --- input DMAs, chunked + ordered by first use ---
            xt_t = xt.rearrange("(kt p) b -> p kt b", p=P)
            w_t = [
                w_aps[l].rearrange("(kt p) m -> p kt m", p=P) for l in range(len(DIMS))
            ]
            b_t = [
                b_aps[l].rearrange("(mt p) -> p mt", p=P) for l in range(len(DIMS))
            ]

            def dma_x(n):
                sl = slice(n * BN, (n + 1) * BN)
                nc.sync.dma_start(x_sb[:, :, sl], xt_t[:, :, sl])

            def dma_w(l, half):
                m = DIMS[l][1]
                sl = slice(half * (m // 2), (half + 1) * (m // 2))
                nc.sync.dma_start(w_sb[l][:, :, sl], w_t[l][:, :, sl])

            def dma_b(l):
                nc.sync.dma_start(b_sb[l][:], b_t[l])

            dma_x(0)
            dma_w(0, 0)
            dma_x(1)
            dma_w(0, 1)
            dma_b(0)
            dma_w(1, 0)
            dma_b(1)
            dma_w(1, 1)
            dma_w(2, 0)
            dma_b(2)
            dma_w(2, 1)
            dma_w(3, 0)
            dma_b(3)
            dma_w(3, 1)

            def relu_store(idx, dst, ps, bias, func):
                # bias(+relu) from PSUM into SBUF, alternating engines
                if idx % 2 == 0:
                    nc.scalar.activation(dst, ps, func, bias=bias)
                else:
                    if func == mybir.ActivationFunctionType.Relu:
                        nc.vector.tensor_scalar(
                            dst, ps, bias, 0.0,
                            mybir.AluOpType.add, mybir.AluOpType.max,
                        )
                    else:
                        nc.vector.tensor_scalar(
                            dst, ps, bias, None, mybir.AluOpType.add
                        )

            relu = mybir.ActivationFunctionType.Relu
            ident = mybir.ActivationFunctionType.Identity

            def dummy_mms(count):
                # PE filler while DMAs stream in: keeps the array warm,
                # no data dependencies
                for _ in range(count):
                    nc.tensor.matmul(
                        warm_ps[:], warm_src[:, :P], warm_src[:],
                        start=True, stop=True,
                    )

            def layer_chunk(l, n, idx0, ms=None):
                # m-groups of layer l on batch chunk n
                src = x_sb if l == 0 else h_sb[l - 1]
                last = l == len(DIMS) - 1
                kt, mt = DIMS[l][0] // P, DIMS[l][1] // P
                bsl = slice(n * BN, (n + 1) * BN)
                for m in ms if ms is not None else range(mt):
                    bias = b_sb[l][:, m : m + 1]
                    ps = psum.tile([P, BN], F32, tag="ps", name="ps")
                    for k in range(kt):
                        nc.tensor.matmul(
                            ps[:],
                            w_sb[l][:, k, m * P : (m + 1) * P],
                            src[:, k, bsl],
                            start=(k == 0),
                            stop=(k == kt - 1),
                        )
                    if last:
                        o = opool.tile([P, BN], F32, tag="o", name="o")
                        relu_store(idx0 + m, o[:], ps[:], bias, ident)
                        nc.sync.dma_start(yt_t[:, m, bsl], o[:])
                    else:
                        relu_store(idx0 + m, h_sb[l][:, m, bsl], ps[:], bias, relu)
                    if l == 0 and n == 0 and m == 1:
                        dummy_mms(fill1)
                if l == 0 and n == 0 and (ms is None or ms[-1] == mt - 1):
                    dummy_mms(fill2)

            # L0-n1 groups before L0-n0's m2/m3: x1 lands before w0's
            # second half, keeping the PE stream aligned with DMA arrivals
            if l0_order == 0:
                layer_chunk(0, 0, 0)
                layer_chunk(0, 1, 0)
            else:
                layer_chunk(0, 0, 0, ms=(0, 1))
                layer_chunk(0, 1, 0, ms=(0, 1))
                layer_chunk(0, 0, 0, ms=(2, 3))
                layer_chunk(0, 1, 0, ms=(2, 3))
            layer_chunk(1, 0, 1)
            layer_chunk(1, 1, 0)
            layer_chunk(2, 0, 1)
            if l3_interleave:
                layer_chunk(3, 0, 0)
                layer_chunk(2, 1, 0)
                layer_chunk(3, 1, l3_parity)
            else:
                layer_chunk(2, 1, 0)
                layer_chunk(3, 0, l3_parity)
                layer_chunk(3, 1, l3_parity)
    nc.compile()
    return nc


def kernel(_trace=False, **inputs):
    global _cached_nc, LAST_RESULTS
    x = np.ascontiguousarray(inputs["x"], dtype=np.float32)
    if _cached_nc is None:
        _cached_nc = _build()
    nc = _cached_nc

    xt = np.ascontiguousarray(x.T)
    out_sizes = [HID, HID, HID, OUT]
    in_sizes = [IN, HID, HID, HID]
    in_maps = []
    for e in range(N_PAR):
        m = {"xt": xt}
        for l in range(4):
            r0, c0 = e * out_sizes[l], e * in_sizes[l]
            blk = inputs[f"W{l}"][r0 : r0 + out_sizes[l], c0 : c0 + in_sizes[l]]
            m[f"w{l}t"] = np.ascontiguousarray(np.asarray(blk).T, dtype=np.float32)
            m[f"b{l}"] = np.ascontiguousarray(
                np.asarray(inputs[f"b{l}"][r0 : r0 + out_sizes[l]]), dtype=np.float32
            )
        in_maps.append(m)

    res = run_bass_kernel_spmd(
        nc, in_maps, core_ids=list(range(N_PAR)), trace=_trace
    )
    LAST_RESULTS = res
    y_p = np.concatenate(
        [res.results[e]["yt"].T for e in range(N_PAR)], axis=1
    ).astype(np.float32)
    x_p = np.tile(x, (1, N_PAR)).astype(np.float32)
    return (y_p, x_p)
